# revision 70
# baseline (speedup 1.0000x reference)
"""Causal multi-head attention with RoPE (faithful to reference's cos<-sin
overwrite bug) on 8 TRN2 NeuronCores.

Sharding: data parallel on batch (2) x tensor parallel on heads (4 groups of
4 heads) = 8 cores. Each core computes, for its (batch, 4 heads), flash-style
causal attention and its partial out-projection; the host sums the 4 partials
per batch and adds the bias.

Structure (v5):
- RoPE's rotate-half is folded into Wq/Wk on the host; on-device rope is one
  elementwise multiply by a sin table, written directly as fp8e4.
- Q/K projections run in fp8e4 MatmulPerfMode.DoubleRow (half cost): the host
  ships x^T and the folded Wq/Wk pre-packed as [128, 4, 2, .] with
  contraction element e = 256*ci + 128*j + p.
- Scores are computed transposed (S^T[k, q]) per 128-row k-chunk over 512-wide
  q supertiles, BOTH heads of a pair into one [128, 2, 512] PSUM tile; exp
  (the binding resource: ACT is the only engine with exp) runs once per chunk
  over the live region of both heads. Supertile 0/1 scores read the fp8 rope
  output directly; supertile 2/3 scores run fp8 DoubleRow (half cost) on
  [32, s, j, pair, ctx]-packed q/k produced by 16 partition-fold SBUF DMAs.
  The causal mask multiply runs on the idle Pool engine.
- A/V accumulates O[q, d] per 128-q tile for both heads of a pair into one
  [128, 2, 66] PSUM tile; the ones column in V yields row-sums. Finish is a
  DVE f32 copy + [128,2] reciprocal, with the per-head normalize multiply on
  Pool (broadcast tensor_tensor); the last supertile's pair-1 tiles use an
  all-DVE fast path to shorten the post-exp tail.
- Normalized O tiles are pair-transposed on the PE into [d, q] for the
  out-projection (pair-0 transposes run a half-supertile early); z is staged
  in SBUF, two tiles per DMA, with the last four tiles split into 512-wide
  halves (DVE+ACT evictions) and DMA'd individually.
- Emission is software-pipelined around the exp cadence: per chunk slot the
  PE stream is [scores] -> [pinned projections] -> [backlog units] ->
  [deferred A/V of the previous tile]. Backlog units carry an estimated
  DMA-readiness slot so a too-early pop cannot head-of-line-block the
  in-order PE stream, and idempotent ensure() guards keep producer units
  emitted before any reader (Tile dependency tracking is emission-ordered).
"""

import contextlib

import numpy as np
import ml_dtypes

import concourse.bass as bass
import concourse.mybir as mybir
import concourse.tile as tile
from concourse.bass_utils import run_bass_kernel_spmd

BATCH, CTX, ED = 2, 2048, 1024
NH, HD = 16, 64
ROPE_BASE = 10000.0
P = 128
NCORES = 8
HPC = 4  # heads per core
SS = 512  # q supertile width
NJ = CTX // SS  # 4 supertiles
NKT = CTX // P  # 16 k-chunks / ctx tiles
NEC = ED // P  # 8

F32 = mybir.dt.float32
BF16 = mybir.dt.bfloat16
FP8 = mybir.dt.float8e4


def _split_multi_waits(nc, max_waits=1):
    """Walrus in this container rejects >1 sync wait per instruction; hoist
    extra waits onto preceding same-engine NoOps (semantically identical:
    engines execute their stream in order)."""
    n = 0
    for func in nc.m.functions:
        for bb in func.blocks:
            insts = list(bb.instructions)
            out = []
            changed = False
            for inst in insts:
                si = inst.sync_info
                if si and si.on_wait and len(si.on_wait) > max_waits:
                    waits = list(si.on_wait)
                    for k, w in enumerate(waits[:-max_waits]):
                        nop = mybir.InstNoOp(
                            name=f"{inst.name}-ws{k}",
                            sync_info=mybir.SyncInfo(on_wait=[w], on_update=[]),
                        )
                        nop.engine = inst.engine
                        out.append(nop)
                        n += 1
                    inst.sync_info = mybir.SyncInfo(
                        on_wait=waits[-max_waits:], on_update=list(si.on_update or [])
                    )
                    changed = True
                out.append(inst)
            if changed:
                bb.instructions = out
    return n


def _emit(nc, xT, x8, w8q, w8k, wv, wo, sin2, tri2, ident, z, tc):
    Exp = mybir.ActivationFunctionType.Exp
    MULT = mybir.AluOpType.mult
    DR = mybir.MatmulPerfMode.DoubleRow

    with contextlib.ExitStack() as ctx:
        pers = ctx.enter_context(tc.tile_pool(name="pers", bufs=1))
        ptp = ctx.enter_context(tc.tile_pool(name="ptp", bufs=24))
        work = ctx.enter_context(tc.tile_pool(name="work", bufs=2))
        psum = ctx.enter_context(tc.tile_pool(name="psum", bufs=1, space="PSUM"))

        xt_sb = pers.tile([P, NEC, CTX], BF16, tag="xt")
        x8_sb = pers.tile([P, 4, 2, CTX], FP8, tag="x8")
        w8q_sb = pers.tile([P, 4, 2, 256], FP8, tag="w8q")
        w8k_sb = pers.tile([P, 4, 2, 256], FP8, tag="w8k")
        wv_sb = pers.tile([P, NEC, 256], BF16, tag="wv")
        wo_sb = pers.tile([P, 2, ED], BF16, tag="wo")
        sin_sb = pers.tile([P, CTX], BF16, tag="sin")
        tri_sb = pers.tile([P, 2, P], BF16, tag="tri")
        id_sb = pers.tile([P, P], BF16, tag="id")
        v_sb = pers.tile([P, NKT, HPC, 66], BF16, tag="v")
        ot_sb = pers.tile([P, 2, CTX], BF16, tag="ot")
        # rope output is stored fp8 (precision-checked on host: L2 err
        # 1.5e-2 < 2e-2): supertile 0/1 scores read it directly (fp8 cost ==
        # bf16), supertile 2/3 scores use the DoubleRow repack below at half
        # cost.
        qt_sb = pers.tile([P, 2, CTX], FP8, tag="qt")
        kt_sb = pers.tile([P, 2, CTX], FP8, tag="kt")
        # DR-packed copies: [32, s, j, pair, ctx] with d = 32*j + p
        q8d = pers.tile([32, 2, 2, 2, CTX], FP8, tag="q8d")
        k8d = pers.tile([32, 2, 2, 2, CTX], FP8, tag="k8d")

        def ld(dst, src):
            nc.sync.dma_start(dst, src)

        # ---- load batch A: what the first two supertiles' projections and
        # the early V projections need; the rest loads after the early
        # relayout DMAs are emitted (SP processes its stream in order) ----
        ld(w8q_sb[:], w8q)
        ld(x8_sb[:, 0:2, :, 0:512], x8[:, 0:2, :, 0:512])
        ld(sin_sb[:, 0:512], sin2[:, 0:512])
        ld(w8k_sb[:], w8k)
        ld(x8_sb[:, 2:4, :, 0:512], x8[:, 2:4, :, 0:512])
        for c2 in range(2):
            ld(
                wv_sb[:, 4 * c2 : 4 * c2 + 4, :],
                wv[512 * c2 : 512 * (c2 + 1), :].rearrange("(c p) n -> p c n", p=P),
            )
        ld(x8_sb[:, :, :, 512:2048], x8[:, :, :, 512:2048])
        ld(sin_sb[:, 512:1024], sin2[:, 512:1024])
        ld(xt_sb[:, 0:4, 0:512], xT[0:512, 0:512].rearrange("(c p) n -> p c n", p=P))
        ld(xt_sb[:, 4:8, 0:512], xT[512:1024, 0:512].rearrange("(c p) n -> p c n", p=P))
        ld(tri_sb[:], tri2)
        ld(id_sb[:], ident)
        ld(sin_sb[:, 1024:2048], sin2[:, 1024:2048])
        ld(wo_sb[:], wo.rearrange("(cc p) n -> p cc n", p=P))
        ld(
            xt_sb[:, :, 512:1024],
            xT[:, 512:1024].rearrange("(c p) n -> p c n", p=P),
        )
        ld(
            xt_sb[:, :, 1024:1536],
            xT[:, 1024:1536].rearrange("(c p) n -> p c n", p=P),
        )
        ld(
            xt_sb[:, :, 1536:2048],
            xT[:, 1536:2048].rearrange("(c p) n -> p c n", p=P),
        )
        nc.gpsimd.memset(v_sb[:, :, :, 64:65], 1.0)
        # preload the Exp activation table while DMAs stream (the first real
        # exp otherwise pays the ~1.3us table load on the critical path)
        scratch = pers.tile([P, 8], F32, tag="scratch")
        nc.vector.memset(scratch[:], 0.0)
        nc.scalar.activation(scratch[:], scratch[:], Exp, scale=0.125)

        # ---- Q/K projection (fp8 DoubleRow) + rope for one (which, pair,
        # supertile); relayout DMAs repack rope output into [32, 2, .] ----
        def qk_proj(which, p, j5, ptag="st"):
            w8 = w8q_sb if which == "q" else w8k_sb
            ps = psum.tile([P, 2, SS], F32, tag="st", bufs=2, name="psqk")
            psv = ps[:, 0, :]
            c0 = j5 * SS
            for ci in range(4):
                nc.tensor.matmul(
                    psv,
                    lhsT=w8[:, ci, :, p * P : (p + 1) * P],
                    rhs=x8_sb[:, ci, :, c0 : c0 + SS],
                    start=(ci == 0),
                    stop=(ci == 3),
                    perf_mode=DR,
                )
            dst = qt_sb if which == "q" else kt_sb
            nc.vector.tensor_tensor(
                out=dst[:, p, c0 : c0 + SS],
                in0=psv,
                in1=sin_sb[:, c0 : c0 + SS],
                op=MULT,
            )

        # ---- V projection (bf16, natural layout, per ctx tile) ----
        def v_proj(t):
            ps = psum.tile([P, 256], F32, tag="aux", bufs=2, name="psv")
            for c in range(NEC):
                nc.tensor.matmul(
                    ps[:],
                    lhsT=xt_sb[:, c, t * P : (t + 1) * P],
                    rhs=wv_sb[:, c, :],
                    start=(c == 0),
                    stop=(c == NEC - 1),
                )
            nc.vector.tensor_copy(
                out=v_sb[:, t, :, 0:64],
                in_=ps[:].rearrange("p (h d) -> p h d", h=HPC),
            )

        # ---- DR repack: qt/kt [128(dd), pair, ctx] -> [32, s, j, pair, ctx]
        # via 4 partition-block SBUF DMAs (dd = 64s + 32j + p -> d = 32j+p) --
        def dr_repack(which, p):
            src = qt_sb if which == "q" else kt_sb
            dst = q8d if which == "q" else k8d
            for s in range(2):
                for j in range(2):
                    b = 64 * s + 32 * j
                    ld(dst[:, s, j, p, :], src[b : b + 32, p, :])

        # ---- scores+exp(+mask) for both heads of (pair, supertile, chunk) --
        def scores(p, j5, KT):
            st = psum.tile([P, 2, SS], F32, tag="st", bufs=2)
            lo = max(KT * P - j5 * SS, 0)
            for s in range(2):
                if j5 >= 2:  # DoubleRow fp8: half PE cost
                    nc.tensor.matmul(
                        st[:, s, lo:SS],
                        lhsT=k8d[:, s, :, p, KT * P : (KT + 1) * P],
                        rhs=q8d[:, s, :, p, j5 * SS + lo : (j5 + 1) * SS],
                        start=True,
                        stop=True,
                        perf_mode=DR,
                    )
                else:
                    nc.tensor.matmul(
                        st[:, s, lo:SS],
                        lhsT=kt_sb[s * HD : (s + 1) * HD, p, KT * P : (KT + 1) * P],
                        rhs=qt_sb[s * HD : (s + 1) * HD, p, j5 * SS + lo : (j5 + 1) * SS],
                        start=True,
                        stop=True,
                    )
            pt = ptp.tile([P, 2, SS], BF16, tag="pt")
            nc.scalar.activation(pt[:, :, lo:SS], st[:, :, lo:SS], Exp, scale=0.125)
            if KT >= 4 * j5:
                d = KT * P - j5 * SS
                # causal mask of the diagonal block, both heads in one op on
                # the otherwise-idle Pool engine (tri2 holds two copies)
                nc.gpsimd.tensor_tensor(
                    out=pt[:, :, d : d + P],
                    in0=pt[:, :, d : d + P],
                    in1=tri_sb[:],
                    op=MULT,
                )
            return pt

        # ---- A/V for one (pair, supertile, local q tile) in O[q,d] form.
        # Both heads accumulate into one [P, 2, 66] psum tile; eviction is a
        # single DVE f32 copy to SBUF, and the rowsum normalize (divide by
        # the ones-column) runs on the idle Pool/GPSIMD engine via
        # normalize_recip, freeing ~23us of DVE time. ----
        def av_finish(p, o_ap, osb_t, fast=False):
            rc2 = work.tile([P, 2], F32, tag="rc2", bufs=4)
            if fast:
                # latency-optimized tail path: all-DVE, no Pool hop
                nc.vector.reciprocal(rc2[:], o_ap[:, :, 64:65])
                for s in range(2):
                    nc.vector.tensor_scalar(
                        out=osb_t[:, 2 * p + s, :],
                        in0=o_ap[:, s, 0:64],
                        scalar1=rc2[:, s : s + 1],
                        scalar2=None,
                        op0=MULT,
                    )
                return
            oraw = work.tile([P, 2, 65], F32, tag="oraw", bufs=4)
            nc.vector.tensor_copy(out=oraw[:], in_=o_ap)
            nc.vector.reciprocal(rc2[:], oraw[:, :, 64:65])
            for s in range(2):
                nc.gpsimd.tensor_tensor(
                    out=osb_t[:, 2 * p + s, :],
                    in0=oraw[:, s, 0:64],
                    in1=rc2[:, s : s + 1].broadcast_to([P, 64]),
                    op=MULT,
                )

        def av(p, j5, tl, pts, osb_t, fast=False):
            T = 4 * j5 + tl
            o = psum.tile([P, 2, 66], F32, tag="small", bufs=2)
            for s in range(2):
                for KT in range(T + 1):
                    nc.tensor.matmul(
                        o[:, s, 0:65],
                        lhsT=pts[KT][:, s, tl * P : (tl + 1) * P],
                        rhs=v_sb[:, KT, 2 * p + s, 0:65],
                        start=(KT == 0),
                        stop=(KT == T),
                    )
            av_finish(p, o[:, :, 0:65], osb_t, fast=fast)

        # split variant for the last supertile: chunks 0..11 pre-accumulate
        # into a pair-merged 4-tile psum right after chunk 11, so only the
        # diagonal chunks remain on each tile's tail
        def av_pre_j3(p, pts, o4s, tls):
            # the two per-s accumulators borrow the "small" slots, which are
            # idle during supertile-3 chunks >= 12
            if not o4s:
                o4s[0] = psum.tile([P, 4, 66], F32, tag="small", bufs=2,
                                   name=f"o4a{p}")
                o4s[1] = psum.tile([P, 4, 66], F32, tag="small", bufs=2,
                                   name=f"o4b{p}")
            for s in range(2):
                for tl in tls:
                    for KT in range(12):
                        nc.tensor.matmul(
                            o4s[s][:, tl, 0:65],
                            lhsT=pts[KT][:, s, tl * P : (tl + 1) * P],
                            rhs=v_sb[:, KT, 2 * p + s, 0:65],
                            start=(KT == 0),
                            stop=False,
                        )

        def av_j3_tail(p, tl, pts, o4s, osb_t):
            T = 12 + tl
            for s in range(2):
                for KT in range(12, T + 1):
                    nc.tensor.matmul(
                        o4s[s][:, tl, 0:65],
                        lhsT=pts[KT][:, s, tl * P : (tl + 1) * P],
                        rhs=v_sb[:, KT, 2 * p + s, 0:65],
                        start=False,
                        stop=(KT == T),
                    )
            rc2 = work.tile([P, 2], F32, tag="rc2", bufs=4)
            for s in range(2):
                nc.vector.reciprocal(rc2[:, s : s + 1], o4s[s][:, tl, 64:65])
                if p == 1:  # latency-optimized all-DVE tail
                    nc.vector.tensor_scalar(
                        out=osb_t[:, 2 * p + s, :],
                        in0=o4s[s][:, tl, 0:64],
                        scalar1=rc2[:, s : s + 1],
                        scalar2=None,
                        op0=MULT,
                    )
            if p == 0:
                oraw = work.tile([P, 2, 64], F32, tag="oraw", bufs=4)
                nc.vector.tensor_copy(
                    out=oraw[:, 0, :], in_=o4s[0][:, tl, 0:64]
                )
                nc.vector.tensor_copy(
                    out=oraw[:, 1, :], in_=o4s[1][:, tl, 0:64]
                )
                for s in range(2):
                    nc.gpsimd.tensor_tensor(
                        out=osb_t[:, 2 * p + s, :],
                        in0=oraw[:, s, :],
                        in1=rc2[:, s : s + 1].broadcast_to([P, 64]),
                        op=MULT,
                    )

        # ---- pair transpose of normalized O into [d, q] for out_proj ----
        def transpose_pair(cc, T, osb_t, ptag="aux"):
            bufs = {"small": 2, "st": 2, "aux": 2}[ptag]
            tr = psum.tile([P, P], BF16, tag=ptag, bufs=bufs)
            nc.tensor.transpose(tr[:], osb_t[:, 2 * cc : 2 * cc + 2, :], id_sb[:])
            nc.vector.tensor_copy(out=ot_sb[:, cc, T * P : (T + 1) * P], in_=tr[:])

        # ---- out projection, one 512-wide half per call so it can be
        # interleaved between score chunks; z staged 2 tiles per DMA
        # (last two tiles individually to shorten the tail) ----
        zstage = {}

        def out_proj_half(T, nh):
            zp = psum.tile([P, 512], F32, tag="aux", bufs=2, name=f"zp{T}_{nh}")
            for cc in (0, 1):
                nc.tensor.matmul(
                    zp[:],
                    lhsT=ot_sb[:, cc, T * P : (T + 1) * P],
                    rhs=wo_sb[:, cc, nh * 512 : (nh + 1) * 512],
                    start=(cc == 0),
                    stop=(cc == 1),
                )
            if T % 2 == 0 and nh == 0:
                zstage[T // 2] = work.tile(
                    [P, 2, ED], BF16, tag="zs", bufs=3, name=f"zs{T}"
                )
            zs_t = zstage[T // 2]
            if T >= NKT - 3 and nh == 1:
                # tail tiles: second half evicts on ACT (idle after the last
                # exp), in parallel with DVE's first half
                nc.scalar.activation(
                    zs_t[:, T % 2, 512:1024], zp[:],
                    mybir.ActivationFunctionType.Copy,
                )
            else:
                nc.vector.tensor_copy(
                    out=zs_t[:, T % 2, nh * 512 : (nh + 1) * 512], in_=zp[:]
                )
            if T >= NKT - 4:  # tail tiles: DMA each half right after evict
                ld(
                    z[T * P : (T + 1) * P, nh * 512 : (nh + 1) * 512],
                    zs_t[:, T % 2, nh * 512 : (nh + 1) * 512],
                )
            elif nh == 1 and T % 2 == 1:
                ld(
                    z[(T - 1) * P : (T + 1) * P, :].rearrange(
                        "(a p) n -> p a n", p=P
                    ),
                    zs_t[:],
                )

        # ---- emission schedule: passes (j5 ascending, pair inner) ----
        fillers = []

        def pull_filler():
            if fillers:
                fillers.pop(0)()

        # pre-phase: pair0/supertile0 projections on the idle scores psum
        qk_proj("q", 0, 0, ptag="st")
        qk_proj("k", 0, 0, ptag="st")
        # Remaining units flow through a shared backlog ordered by estimated
        # DMA-readiness; each entry is (eligible_slot, fn) and the pump only
        # pops entries whose inputs should have landed, so a too-early pop
        # can't head-of-line-block the in-order PE stream. pass_fillers pins
        # a pass's own score dependencies as a backstop (units are
        # idempotent via the done-set).
        done_units = set()
        unit_reg = {}

        def unit(key, fn):
            def run():
                if key not in done_units:
                    done_units.add(key)
                    fn()
            unit_reg[key] = run
            return run

        def ensure(key):
            # emission-order dependency guard: emit the producer unit now
            # (idempotent) so Tile sees the write before any read of it
            unit_reg[key]()

        vp = lambda t: unit(("v", t), lambda t=t: v_proj(t))
        qk = lambda w, p, j: unit((w, p, j), lambda: qk_proj(w, p, j))
        # the pre-phase emitted pair-0/supertile-0 projections directly
        done_units.update({("q", 0, 0), ("k", 0, 0)})
        unit_reg[("q", 0, 0)] = unit_reg[("k", 0, 0)] = lambda: None

        def _rp(w, p):
            for j in range(4):  # repack reads all four supertiles' ropes
                ensure((w, p, j))
            dr_repack(w, p)

        rp = lambda w, p: unit(("r", w, p), lambda w=w, p=p: _rp(w, p))
        pass_fillers = {
            (0, 0): [qk("q", 1, 0), qk("k", 1, 0)],
            (0, 1): [qk("q", 0, 1), qk("k", 0, 1),
                     qk("q", 1, 1), qk("k", 1, 1)],
            (1, 0): [qk("q", 0, 2), qk("k", 0, 2),
                     qk("q", 1, 2), qk("k", 1, 2)],
            (1, 1): [qk("q", 0, 3), qk("k", 0, 3),
                     qk("q", 1, 3), qk("k", 1, 3),
                     rp("q", 0), rp("k", 0), rp("q", 1), rp("k", 1)],
        }
        # (eligible_slot, unit): slots estimated from serialized input-DMA
        # completion times vs the ~1us/chunk exp cadence
        fillers.extend([(0, qk("q", 1, 0)), (0, qk("k", 1, 0))])
        fillers.extend([(4, qk("q", 0, 1)), (4, qk("k", 0, 1)),
                        (5, qk("q", 1, 1)), (5, qk("k", 1, 1))])
        fillers.extend([(6, vp(t)) for t in range(0, 4)])
        fillers.extend([(8, qk("q", 0, 2)), (8, qk("k", 0, 2)),
                        (9, qk("q", 0, 3)), (9, qk("k", 0, 3)),
                        (10, qk("q", 1, 2)), (10, qk("k", 1, 2)),
                        (11, qk("q", 1, 3)), (11, qk("k", 1, 3))])
        fillers.extend([(11, vp(t)) for t in range(4, 8)])
        fillers.extend([(14, rp("q", 0)), (14, rp("k", 0)),
                        (15, rp("q", 1)), (15, rp("k", 1))])
        fillers.extend([(16, vp(t)) for t in range(8, 12)])
        fillers.extend([(19, vp(t)) for t in range(12, 16)])

        # Software-pipelined emission: per chunk slot the PE stream is
        # [scores KT] -> [a few backlog units] -> [deferred av of tile KT-1],
        # so the ACT engine's exp stream — the binding resource — runs
        # back-to-back. A/V runs at one-chunk lag (bounded PSUM life);
        # transposes and out-projection halves go into a work-conserving
        # backlog drained evenly over the remaining chunk slots, crossing
        # pass boundaries.
        osb = {}
        deferred_av = []
        backlog = fillers  # projections already queued; tile units append

        total_slots = 2 * sum(4 * (j5 + 1) for j5 in range(NJ))
        slot = 0

        def pump(remaining_local):
            # drain backlog at a rate that finishes by the end of emission,
            # skipping entries whose estimated input-DMA hasn't landed yet
            rem_slots = max(total_slots - slot, 1)
            n = (len(backlog) + rem_slots - 1) // rem_slots
            if remaining_local:
                n = max(n - 1, 0)
            for _ in range(n):
                for i, (elig, fn) in enumerate(backlog):
                    if elig <= slot:
                        backlog.pop(i)
                        fn()
                        break
                else:
                    break

        def ttag_for(T, j5):
            if j5 < 3:
                return "small"
            if T in (13, 14):
                return "st"
            return "aux"

        def consume_tile(p, j5, tl, pts, T, o4s):
            for t in range(T + 1):  # A/V reads v chunks 0..T
                ensure(("v", t))
            av(p, j5, tl, pts, osb[T], fast=(j5 == 3 and p == 1))
            # pair-0 transpose only needs pair-0's osb half: run it during
            # the p=0 pass so the p=1 tail only waits on transpose_pair(1)
            if p == 0:
                backlog.append(
                    (0, lambda: transpose_pair(0, T, osb[T], ptag=ttag_for(T, j5)))
                )
            if p == 1:
                backlog.append(
                    (0, lambda: transpose_pair(1, T, osb[T], ptag=ttag_for(T, j5)))
                )
                backlog.append((0, lambda: out_proj_half(T, 0)))
                backlog.append((0, lambda: out_proj_half(T, 1)))

        for j5 in range(NJ):
            for p in (0, 1):
                local = list(pass_fillers.get((j5, p), []))
                pts = {}
                o4s = {}
                for KT in range(4 * (j5 + 1)):
                    if j5 >= 2 and KT == 0:
                        # DR scores read the repacked q/k: backstop-emit
                        ensure(("r", "q", p))
                        ensure(("r", "k", p))
                    pts[KT] = scores(p, j5, KT)
                    slot += 1
                    if local:
                        local.pop(0)()
                    pump(bool(local))
                    while deferred_av:
                        deferred_av.pop(0)()
                    tl = KT - 4 * j5
                    if tl >= 0:
                        T = KT
                        if p == 0:
                            osb[T] = work.tile(
                                [P, HPC, 64], BF16, tag="osb", bufs=12, name=f"osb{T}"
                            )
                        fn = (
                            lambda p=p, j5=j5, tl=tl, pts=pts, T=T, o4s=o4s:
                            consume_tile(p, j5, tl, pts, T, o4s)
                        )
                        if j5 == 0:
                            # supertile-0 A/V waits on the V projections
                            # (wv/xt DMAs ~slot 7); backlog-gate it instead
                            # of stalling the PE stream at the next chunk
                            backlog.append((7 + p, fn))
                        else:
                            deferred_av.append(fn)
        while deferred_av:
            deferred_av.pop(0)()
        while backlog:
            backlog.pop(0)[1]()


def _build_program(split_waits=True):
    nc = bass.Bass("TRN2", target_bir_lowering=False, debug=False, num_devices=NCORES)
    xT = nc.dram_tensor("xT", [ED, CTX], BF16, kind="ExternalInput").ap()
    x8 = nc.dram_tensor("x8", [P, 4, 2, CTX], FP8, kind="ExternalInput").ap()
    w8q = nc.dram_tensor("w8q", [P, 4, 2, 256], FP8, kind="ExternalInput").ap()
    w8k = nc.dram_tensor("w8k", [P, 4, 2, 256], FP8, kind="ExternalInput").ap()
    wv = nc.dram_tensor("wv", [ED, 256], BF16, kind="ExternalInput").ap()
    wo = nc.dram_tensor("wo", [256, ED], BF16, kind="ExternalInput").ap()
    sin2 = nc.dram_tensor("sin2", [P, CTX], BF16, kind="ExternalInput").ap()
    tri2 = nc.dram_tensor("tri2", [P, 2 * P], BF16, kind="ExternalInput").ap()
    ident = nc.dram_tensor("ident", [P, P], BF16, kind="ExternalInput").ap()
    z = nc.dram_tensor("z", [CTX, ED], BF16, kind="ExternalOutput").ap()
    with tile.TileContext(nc) as tc:
        _emit(nc, xT, x8, w8q, w8k, wv, wo, sin2, tri2, ident, z, tc)
    if split_waits:
        _split_multi_waits(nc)
    return nc


_PROGRAM = None


def _get_program():
    global _PROGRAM
    if _PROGRAM is None:
        _PROGRAM = _build_program()
    return _PROGRAM


def _host_tables():
    # rotate-half fold matrix: q_rot = R q
    Rm = np.zeros((HD, HD), np.float32)
    for i in range(HD // 2):
        Rm[i, i] = 1.0
        Rm[i, i + 32] = -1.0
        Rm[i + 32, i + 32] = 1.0
        Rm[i + 32, i] = 1.0
    j = np.arange(HD // 2, dtype=np.float32)
    thetas = 1.0 / ROPE_BASE ** (2.0 * j / (HD // 2))
    pos = np.arange(CTX, dtype=np.float32)
    ang = pos[:, None] * thetas[None, :]
    sinT = np.sin(np.concatenate([ang, ang], axis=-1)).T.astype(np.float32)  # [64,CTX]
    sin2 = np.ascontiguousarray(np.tile(sinT, (2, 1))).astype(
        ml_dtypes.bfloat16
    )  # [128, CTX]
    cg = np.arange(P)[None, :]
    ii = np.arange(P)[:, None]
    tri = (cg >= ii).astype(np.float32)  # keep q >= k
    tri2 = np.ascontiguousarray(np.concatenate([tri, tri], axis=1)).astype(
        ml_dtypes.bfloat16
    )  # [128, 256]
    ident = np.eye(P, dtype=np.float32).astype(ml_dtypes.bfloat16)
    return Rm, sin2, tri2, ident


def _pack_dr(a):
    """[1024, n] -> [128, 4, 2, n] fp8 with element e = 256*ci + 128*j + p."""
    n = a.shape[1]
    return np.ascontiguousarray(
        a.reshape(4, 2, P, n).transpose(2, 0, 1, 3)
    ).astype(ml_dtypes.float8_e4m3)


def _run(x, Wq, Wk, Wv, Wo):
    nc = _get_program()
    Rm, sin2, tri2, ident = _host_tables()

    def fold(W):
        W2 = W.reshape(ED, NH, HD)
        return np.einsum("enh,gh->eng", W2, Rm).reshape(ED, NH * HD)

    bf = ml_dtypes.bfloat16
    Wq_f = fold(Wq)
    Wk_f = fold(Wk)
    Wv_b = Wv.astype(bf)
    Wo_b = Wo.astype(bf)
    xT_f = [np.ascontiguousarray(x[b].T) for b in range(BATCH)]
    x8_b = [_pack_dr(t) for t in xT_f]
    xT_b = [t.astype(bf) for t in xT_f]

    in_maps = []
    for core in range(NCORES):
        b, g = core // 4, core % 4
        cs = slice(256 * g, 256 * (g + 1))
        in_maps.append(
            {
                "xT": xT_b[b],
                "x8": x8_b[b],
                "w8q": _pack_dr(np.ascontiguousarray(Wq_f[:, cs])),
                "w8k": _pack_dr(np.ascontiguousarray(Wk_f[:, cs])),
                "wv": np.ascontiguousarray(Wv_b[:, cs]),
                "wo": np.ascontiguousarray(Wo_b[cs, :]),
                "sin2": sin2,
                "tri2": tri2,
                "ident": ident,
            }
        )
    return nc, in_maps


def kernel(x, Wq, Wk, Wv, Wo, bo):
    x = np.asarray(x, dtype=np.float32)
    nc, in_maps = _run(x, np.asarray(Wq, np.float32), np.asarray(Wk, np.float32),
                       np.asarray(Wv, np.float32), np.asarray(Wo, np.float32))
    res = run_bass_kernel_spmd(nc, in_maps, core_ids=list(range(NCORES)))
    out = np.zeros((BATCH, CTX, ED), np.float32)
    for core in range(NCORES):
        b = core // 4
        out[b] += res.results[core]["z"]
    out += np.asarray(bo, np.float32)[None, None, :]
    return out

